# revision 6
# baseline (speedup 1.0000x reference)
"""Trainium2 Bass kernel for batched single-head attention with seq-sum pooling.

Reference computation (B=16, S=2048, D=512, fp32):
    q = x @ W_q ; k = x @ W_k ; v = x @ W_v          per batch  [S, D]
    scores = q @ k.T / sqrt(D)                        [S, S]
    attn = softmax(scores, axis=-1)
    out_b = sum_s (attn @ v)[s, :]                    [D]

Key algebraic restructure: the final sum over query positions commutes with
the attn @ v matmul:
    out_b = (sum_q attn[q, :]) @ v = (r^T E) @ v
where E = exp(scores / sqrt(D)) and r[q] = 1 / rowsum_q(E).  This removes the
second [S,S]x[S,D] matmul (~36% of the FLOPs) and replaces it with one
[1,S]x[S,S] column-sum matmul plus one [1,S]x[S,D] matvec.

Sharding: pure data parallelism over batch — 2 batch elements per core on 8
NeuronCores, weights replicated, no collectives.  Host concatenates per-core
[2, D] outputs.

Matmuls run as float32r (fp32 data, relaxed-precision PE mode, 4x fp32
throughput).  Measured rel error of an f32r matmul chain is ~1.5e-4.
"""

import sys

sys.path.insert(0, "/opt/trn_rl_repo")

import numpy as np

import concourse.bass as bass
import concourse.mybir as mybir
import concourse.tile as tile
from concourse import bacc
from concourse.bass_utils import run_bass_kernel_spmd
from concourse.masks import make_identity

B, S, D = 16, 2048, 512
P = 128
N_CORES = 8
B_PER_CORE = B // N_CORES  # 2
SCALE = 1.0 / float(np.sqrt(D))

F32 = mybir.dt.float32
F32R = mybir.dt.float32r

N_ST = S // P  # 16 s-tiles (partition tiles of the sequence dim)
N_DT = D // P  # 4 d-tiles (partition tiles of the feature dim)
NCH = 512  # moving free dim per matmul (one PSUM bank of fp32)
N_SC = S // NCH  # 4 s-chunks of the sequence dim
N_KC = S // NCH  # 4 k-chunks of the key dim


def build_nc():
    nc = bacc.Bacc("TRN2", target_bir_lowering=False, debug=False, num_devices=N_CORES)
    x_ext = nc.dram_tensor(
        "inputs", [B_PER_CORE, S, D], F32, kind="ExternalInput"
    ).ap()
    wq_ext = nc.dram_tensor("W_q", [D, D], F32, kind="ExternalInput").ap()
    wk_ext = nc.dram_tensor("W_k", [D, D], F32, kind="ExternalInput").ap()
    wv_ext = nc.dram_tensor("W_v", [D, D], F32, kind="ExternalInput").ap()
    out_ext = nc.dram_tensor("out", [B_PER_CORE, D], F32, kind="ExternalOutput").ap()

    with tile.TileContext(nc) as tc:
        with (
            tc.tile_pool(name="const", bufs=1) as const_pool,
            tc.tile_pool(name="w", bufs=1) as w_pool,
            tc.tile_pool(name="xin", bufs=4) as xin_pool,
            tc.tile_pool(name="xt", bufs=1) as xt_pool,
            tc.tile_pool(name="qkv", bufs=1) as qkv_pool,
            tc.tile_pool(name="e", bufs=3) as e_pool,
            tc.tile_pool(name="soft", bufs=4) as soft_pool,
            tc.tile_pool(name="wvec", bufs=1) as wvec_pool,
            tc.tile_pool(name="scps", bufs=2, space="PSUM") as sc_psum,
            tc.tile_pool(name="gpps", bufs=2, space="PSUM") as gp_psum,
            tc.tile_pool(name="wps", bufs=1, space="PSUM") as w_psum,
        ):
            ident = const_pool.tile([P, P], F32)
            make_identity(nc, ident[:])
            one_f = const_pool.tile([1, 2], F32)
            nc.gpsimd.memset(one_f[:], 1.0)
            one_t = const_pool.tile([1, 2], F32R)
            nc.vector.tensor_copy(one_t[:], one_f[:])

            # Weights: [D, D] -> per-contraction-tile layout [P, N_DT, D], f32r
            w_tiles = []
            for name, ext in (("wq", wq_ext), ("wk", wk_ext), ("wv", wv_ext)):
                w_s = w_pool.tile([P, N_DT, D], F32R, tag=name)
                nc.gpsimd.dma_start(
                    out=w_s[:], in_=ext.rearrange("(t p) e -> p t e", p=P)
                )
                w_tiles.append(w_s)
            wq_s, wk_s, wv_s = w_tiles

            def phase_load_transpose(b):
                """DMA x[b] and PE-transpose into xT [d, s] (f32r)."""
                xt_s = xt_pool.tile([P, N_DT, S], F32R, tag="xt")
                for st in range(N_ST):
                    x_tile = xin_pool.tile([P, D], F32, tag="xin")
                    nc.sync.dma_start(
                        out=x_tile[:], in_=x_ext[b, st * P : (st + 1) * P, :]
                    )
                    for dt_i in range(N_DT):
                        tp = gp_psum.tile([P, P], F32, tag="gp")
                        nc.tensor.transpose(
                            tp[:], x_tile[:, dt_i * P : (dt_i + 1) * P], ident[:]
                        )
                        nc.vector.tensor_copy(
                            xt_s[:, dt_i, st * P : (st + 1) * P], tp[:]
                        )
                return xt_s

            def phase_projections(b, xt_s):
                """QT/KT [e, s] and V [s, d] projections, all f32r."""
                qt_s = qkv_pool.tile([P, N_DT, S], F32R, tag="qt")
                kt_s = qkv_pool.tile([P, N_DT, S], F32R, tag="kt")
                v_s = qkv_pool.tile([P, N_ST, D], F32R, tag="v")
                # QT / KT: lhsT = W tile [d, e-chunk], rhs = xT [d, s-chunk]
                for w_src, dst in ((wq_s, qt_s), (wk_s, kt_s)):
                    for et in range(N_DT):
                        for sc in range(N_SC):
                            mp = gp_psum.tile([P, NCH], F32, tag="gp")
                            for kd in range(N_DT):
                                nc.tensor.matmul(
                                    mp[:],
                                    w_src[:, kd, et * P : (et + 1) * P],
                                    xt_s[:, kd, sc * NCH : (sc + 1) * NCH],
                                    start=(kd == 0),
                                    stop=(kd == N_DT - 1),
                                )
                            nc.vector.tensor_copy(
                                dst[:, et, sc * NCH : (sc + 1) * NCH], mp[:]
                            )
                # V natural: lhsT = xT tile [d, s-chunk of 128], rhs = W_v [d, e]
                for st in range(N_ST):
                    mp = gp_psum.tile([P, NCH], F32, tag="gp")
                    for kd in range(N_DT):
                        nc.tensor.matmul(
                            mp[:],
                            xt_s[:, kd, st * P : (st + 1) * P],
                            wv_s[:, kd, :],
                            start=(kd == 0),
                            stop=(kd == N_DT - 1),
                        )
                    nc.vector.tensor_copy(v_s[:, st, :], mp[:])
                return qt_s, kt_s, v_s

            def emit_scores_qt(qt_s, kt_s, qt):
                """scores + exp + rowsum + reciprocal for one q-tile."""
                e_t = e_pool.tile([P, S], F32R, tag="e")
                rsum = soft_pool.tile([P, N_KC], F32, tag="rsum")
                for kc in range(N_KC):
                    sp = sc_psum.tile([P, NCH], F32, tag="sc")
                    for et in range(N_DT):
                        nc.tensor.matmul(
                            sp[:],
                            qt_s[:, et, qt * P : (qt + 1) * P],
                            kt_s[:, et, kc * NCH : (kc + 1) * NCH],
                            start=(et == 0),
                            stop=(et == N_DT - 1),
                        )
                    nc.scalar.activation(
                        e_t[:, kc * NCH : (kc + 1) * NCH],
                        sp[:],
                        mybir.ActivationFunctionType.Exp,
                        scale=SCALE,
                        accum_out=rsum[:, kc : kc + 1],
                    )
                rtot = soft_pool.tile([P, 1], F32, tag="rtot")
                nc.vector.reduce_sum(rtot[:], rsum[:], axis=mybir.AxisListType.X)
                rrec = soft_pool.tile([P, 1], F32, tag="rrec")
                nc.vector.reciprocal(rrec[:], rtot[:])
                # f32r matmuls need a full 128-wide stationary operand; broadcast
                # r across all columns so every PSUM output row equals r^T E.
                r_t = soft_pool.tile([P, P], F32R, tag="r")
                nc.vector.tensor_copy(r_t[:], rrec[:, 0:1].broadcast_to([P, P]))
                return e_t, r_t

            def emit_colsum_qt(w_ps, e_t, r_t, qt):
                """w_ps[:, kc, :] += bcast(r_qt)^T @ E_qt (every row = colsum)."""
                for kc in range(N_KC):
                    nc.tensor.matmul(
                        w_ps[:, kc, :],
                        r_t[:],
                        e_t[:, kc * NCH : (kc + 1) * NCH],
                        start=(qt == 0),
                        stop=(qt == N_ST - 1),
                        skip_group_check=True,
                    )

            def phase_scores(b, qt_s, kt_s):
                w_ps = w_psum.tile([P, N_KC, NCH], F32, tag="w")
                prev = None
                for qt in range(N_ST):
                    cur = emit_scores_qt(qt_s, kt_s, qt)
                    if prev is not None:
                        emit_colsum_qt(w_ps, prev[0], prev[1], qt - 1)
                    prev = cur
                emit_colsum_qt(w_ps, prev[0], prev[1], N_ST - 1)
                return w_ps

            def phase_final(b, w_ps, v_s):
                # w [1, S] to SBUF (f32r, rounded by ACT copy)
                w_sb = wvec_pool.tile([1, S], F32R, tag="wsb")
                for kc in range(N_KC):
                    nc.scalar.copy(
                        w_sb[:, kc * NCH : (kc + 1) * NCH], w_ps[0:1, kc, :]
                    )
                # transpose w -> wT chunks [P, 2] via K=1 matmuls against [1,2]
                # ones (N=2 keeps the f32r even-moving-dim rule), broadcast each
                # chunk across a 128-wide stationary tile for the final matmul.
                # Interleave the transposes with the final accumulation matmuls
                # (4 rotating wt_pad slots) so the PE never waits on a copy.
                o_ps = sc_psum.tile([P, NCH], F32, tag="sc")
                wt_pads = {}

                def emit_final_mm(st):
                    nc.tensor.matmul(
                        o_ps[:],
                        wt_pads[st][:],
                        v_s[:, st, :],
                        start=(st == 0),
                        stop=(st == N_ST - 1),
                        skip_group_check=True,
                    )

                for kt in range(N_ST):
                    tp = gp_psum.tile([P, 2], F32, tag="gp")
                    nc.tensor.matmul(
                        tp[:],
                        w_sb[0:1, kt * P : (kt + 1) * P],
                        one_t[0:1, 0:2],
                        start=True,
                        stop=True,
                    )
                    wt_pad = wvec_pool.tile([P, P], F32R, tag=f"wtp{kt % 4}")
                    nc.vector.tensor_copy(
                        wt_pad[:], tp[:, 0:1].broadcast_to([P, P])
                    )
                    wt_pads[kt] = wt_pad
                    if kt >= 3:
                        emit_final_mm(kt - 3)
                for st in range(N_ST - 3, N_ST):
                    emit_final_mm(st)
                o_sb = wvec_pool.tile([1, NCH], F32, tag="osb")
                nc.scalar.copy(o_sb[:], o_ps[0:1, :])
                nc.sync.dma_start(out=out_ext[b : b + 1, :], in_=o_sb[:])

            # Software-pipelined emission: batch 1's load/transpose fills the
            # PE while batch 0's softmax tail / w-phase dependencies resolve.
            xt0 = phase_load_transpose(0)
            q0, k0, v0 = phase_projections(0, xt0)
            wps0 = phase_scores(0, q0, k0)
            xt1 = phase_load_transpose(1)
            phase_final(0, wps0, v0)
            q1, k1, v1 = phase_projections(1, xt1)
            wps1 = phase_scores(1, q1, k1)
            phase_final(1, wps1, v1)

    nc.compile()
    return nc


_NC_CACHE = None


def _get_nc():
    global _NC_CACHE
    if _NC_CACHE is None:
        _NC_CACHE = build_nc()
    return _NC_CACHE


def make_in_maps(inputs, W_q, W_k, W_v):
    inputs = np.ascontiguousarray(np.asarray(inputs, dtype=np.float32))
    W_q = np.ascontiguousarray(np.asarray(W_q, dtype=np.float32))
    W_k = np.ascontiguousarray(np.asarray(W_k, dtype=np.float32))
    W_v = np.ascontiguousarray(np.asarray(W_v, dtype=np.float32))
    return [
        {
            "inputs": inputs[i * B_PER_CORE : (i + 1) * B_PER_CORE],
            "W_q": W_q,
            "W_k": W_k,
            "W_v": W_v,
        }
        for i in range(N_CORES)
    ]


def kernel(**inputs) -> np.ndarray:
    nc = _get_nc()
    in_maps = make_in_maps(
        inputs["inputs"], inputs["W_q"], inputs["W_k"], inputs["W_v"]
    )
    res = run_bass_kernel_spmd(nc, in_maps, core_ids=list(range(N_CORES)))
    return np.concatenate(
        [res.results[i]["out"] for i in range(N_CORES)], axis=0
    ).astype(np.float32)
